# revision 14
# baseline (speedup 1.0000x reference)
"""BiAttentionLayer Trainium2 kernel (Bass/Tile), data-parallel over batch N.

Full inputs:  H [64,1024,200], U [64,64,200], c_mask [64,1024],
              q_mask [64,64], w [600], b []
Full output:  G [64,1024,800] = concat([H, U_, H*U_, H*H_], -1)

Sharding: batch rows 8 per core across 8 NeuronCores; masks/w/b replicated.

Math (matches the reference to fp rounding):
  S = (H@w_h)[:,:,None] + (U@w_u)[:,None,:] + (H*w_hu)@U^T + b
  masked_softmax(v,m) == exp(v*m)*m / sum_j(exp(v*m)*m).
  e = exp(ps_s - 100) in one ACT op, where the S matmul rhs folds all of
  S + mask + offset:
    uwq1/uwq2 rows d: (U^T[d,j]*w_hu[d] + w_h[d])*qm[j]
    uwq2 row 72:      (U@w_u + b)[j]*qm[j]   (ones row in lhsT)
    uwq2 row 73:      100*qm[j]              (100 is exact in bf16)
  Masked lanes give exp(-100) -> flushes to 0 in bf16: exact masking.
  U_ = (e @ [U|1]) * (1/denom): U16's ones column makes the U_ matmul
  emit the softmax denominator as output column 200.
  H_ = (rt @ [H|1]) with rt = max_j(e)*cm; hbar column 200 is sum_t(rt),
  so the Q2C normalizer needs no reduction chain.

All PE traffic is bf16 (fp32 matmul = 2-pass LOW_HIGH, fp32 transpose =
2 cyc/row; bf16 single-pass). PSUM accumulation stays fp32.

Per-instruction fixed costs dominate (ACT/DVE ~200-300 ns, DMA issue
~650 ns on the issuing engine), so work is batched: H loads and G
stores move 4 chunks (512 t-rows) per DMA issue, and the bf16 H
convert / H*U_ / H*H_ / PSUM drains run on 2- or 4-chunk strided APs.

Pipeline: head(c)=transpose+S-matmul, pair(p)=exp+e-transpose+max+rank1,
tailp(p)=U_ matmul pair+normalize+H*U_, with the previous row's H*H_
and quad stores interleaved; ~4 chunks in flight.
"""

import os
import sys

for _p in ("/opt/trn_rl_repo", "/root/.axon_site/_ro/trn_rl_repo"):
    if os.path.isdir(_p) and _p not in sys.path:
        sys.path.insert(0, _p)

import numpy as np

import concourse.bass as bass
import concourse.tile as tile
from concourse import mybir
from concourse.masks import make_identity

N_CORES = 8
N_FULL = 64
B = N_FULL // N_CORES          # batch rows per core
T = 1024
J = 64
D2 = 200
DG = 4 * D2                    # 800
NCHUNK = T // 128              # 8
NPAIR = NCHUNK // 2            # 4
NQUAD = NCHUNK // 4            # 2
K1, K2 = 128, D2 - 128         # contraction split 128 + 72
NEG_SOFT = 100.0               # exp(x - 100): masked lanes underflow to 0
WB = D2 + 2                    # hb16 per-chunk block width (200 H + 2 ones)

FP = mybir.dt.float32
BF = mybir.dt.bfloat16
MULT = mybir.AluOpType.mult
ADD = mybir.AluOpType.add
AXX = mybir.AxisListType.X
EXP = mybir.ActivationFunctionType.Exp
COPYF = mybir.ActivationFunctionType.Copy


def _split_overwide_waits(nc, max_waits=1):
    """This walrus build only encodes one semaphore wait per instruction;
    hoist extra waits onto no-ops just before the offending instruction."""
    for bb in nc.m.functions[0].blocks:
        i = 0
        while i < len(bb.instructions):
            ins = bb.instructions[i]
            si = getattr(ins, "sync_info", None)
            if si is not None and si.on_wait is not None and len(si.on_wait) > max_waits:
                waits = list(si.on_wait)
                si.on_wait = waits[-max_waits:]
                rest = waits[:-max_waits]
                k = 0
                while rest:
                    chunk, rest = rest[:max_waits], rest[max_waits:]
                    nop = mybir.InstNoOp(
                        name=f"{ins.name}-wsplit{k}",
                        engine=ins.engine,
                        bass_nofuse=True,
                        sync_info=mybir.SyncInfo(on_wait=chunk, on_update=[]),
                    )
                    bb.instructions.insert(i, nop)
                    i += 1
                    k += 1
            i += 1


def build_program(split_waits=True):
    nc = bass.Bass()

    H_d = nc.dram_tensor("H", [B, T, D2], FP, kind="ExternalInput")
    U_d = nc.dram_tensor("U", [B, J, D2], FP, kind="ExternalInput")
    cm_d = nc.dram_tensor("c_mask", [B, T], FP, kind="ExternalInput")
    qm_d = nc.dram_tensor("q_mask", [B, J], FP, kind="ExternalInput")
    w_d = nc.dram_tensor("w", [3 * D2], FP, kind="ExternalInput")
    b_d = nc.dram_tensor("b", [1, 1], FP, kind="ExternalInput")
    G_d = nc.dram_tensor("G", [B, T, DG], FP, kind="ExternalOutput")

    with tile.TileContext(nc) as tc:
        with (
            tc.tile_pool(name="const", bufs=1) as constp,
            tc.tile_pool(name="row", bufs=2) as rowp,
            tc.tile_pool(name="chunk", bufs=2) as chp,
            tc.tile_pool(name="gbuf", bufs=7) as gp,
            tc.tile_pool(name="ps_tr", bufs=2, space="PSUM") as ps_trp,
            tc.tile_pool(name="ps_s", bufs=3, space="PSUM") as ps_sp,
            tc.tile_pool(name="ps_u", bufs=2, space="PSUM") as ps_up,
            tc.tile_pool(name="ps_h", bufs=1, space="PSUM") as ps_hp,
        ):
            # ---- constants ----
            ident16 = constp.tile([128, 128], BF)
            make_identity(nc, ident16)
            ones_row16 = constp.tile([1, 128], BF)
            nc.vector.memset(ones_row16, 1.0)
            negc = constp.tile([128, 1], FP)
            nc.vector.memset(negc, -NEG_SOFT)
            b_sb = constp.tile([1, 1], FP)
            nc.gpsimd.dma_start(out=b_sb, in_=b_d[:, :])
            wh1 = constp.tile([K1, 1], FP)
            wh2 = constp.tile([K2, 1], FP)
            wu1 = constp.tile([K1, 1], FP)
            wu2 = constp.tile([K2, 1], FP)
            whu1 = constp.tile([K1, 1], FP)
            whu2 = constp.tile([K2, 1], FP)
            for sb, lo in ((wh1, 0), (wh2, K1), (wu1, D2), (wu2, D2 + K1),
                           (whu1, 2 * D2), (whu2, 2 * D2 + K1)):
                n = sb.shape[0]
                nc.gpsimd.dma_start(out=sb, in_=w_d[lo:lo + n].unsqueeze(1))
            wu16_1 = constp.tile([K1, 1], BF)
            wu16_2 = constp.tile([K2, 1], BF)
            nc.vector.tensor_copy(out=wu16_1, in_=wu1)
            nc.vector.tensor_copy(out=wu16_2, in_=wu2)

            def row_setup(r):
                st = {}
                U_sb = rowp.tile([J, D2], FP, tag="usb")
                nc.sync.dma_start(out=U_sb, in_=U_d[r])
                # U16 col 200 = ones -> U_ matmul also emits the denominator
                U16 = rowp.tile([J, D2 + 1], BF, tag="u16")
                nc.vector.tensor_copy(out=U16[:, 0:D2], in_=U_sb)
                nc.gpsimd.memset(U16[:, D2:D2 + 1], 1.0)
                qm_b = rowp.tile([128, J], FP, tag="qmb")
                nc.gpsimd.dma_start(out=qm_b, in_=qm_d[r].partition_broadcast(128))
                cm_t = rowp.tile([128, NCHUNK], FP, tag="cmt")
                nc.gpsimd.dma_start(
                    out=cm_t, in_=cm_d[r].rearrange("(c p) -> p c", p=128)
                )

                # U^T via PE transpose (two D2 chunks)
                tru = ps_sp.tile([128, 192], BF, tag="s")
                nc.tensor.transpose(tru[0:K1, 0:J], U16[:, 0:K1], ident16[0:J, 0:J])
                nc.tensor.transpose(
                    tru[0:K2, J:2 * J], U16[:, K1:D2], ident16[0:J, 0:J]
                )
                ut1 = rowp.tile([K1, J], BF, tag="ut1")
                ut2 = rowp.tile([K2, J], BF, tag="ut2")
                nc.scalar.copy(out=ut1, in_=tru[0:K1, 0:J])
                nc.scalar.copy(out=ut2, in_=tru[0:K2, J:2 * J])

                # S-matmul rhs: uwq[d, j] = (U^T[d,j]*whu[d] + wh[d]) * qm[j]
                tmp1 = rowp.tile([K1, J], FP, tag="tmp1")
                tmp2 = rowp.tile([K2, J], FP, tag="tmp2")
                nc.vector.tensor_scalar(
                    out=tmp1, in0=ut1, scalar1=whu1[:, 0:1], scalar2=wh1[:, 0:1],
                    op0=MULT, op1=ADD,
                )
                nc.vector.tensor_scalar(
                    out=tmp2, in0=ut2, scalar1=whu2[:, 0:1], scalar2=wh2[:, 0:1],
                    op0=MULT, op1=ADD,
                )
                uwq1 = rowp.tile([K1, J], BF, tag="uwq1")
                uwq2 = rowp.tile([K2 + 2, J], BF, tag="uwq2")
                nc.vector.tensor_tensor(
                    out=uwq1, in0=tmp1, in1=qm_b[0:K1, :], op=MULT
                )
                nc.vector.tensor_tensor(
                    out=uwq2[0:K2, :], in0=tmp2, in1=qm_b[0:K2, :], op=MULT
                )

                # uwq2 row 72 = (U@w_u + b)*qm, row 73 = 100*qm
                ps2 = ps_sp.tile([128, 128], FP, tag="s", name="ps2")
                nc.tensor.matmul(ps2[0:J, 0:1], ut1, wu16_1, start=True, stop=False)
                nc.tensor.matmul(ps2[0:J, 0:1], ut2, wu16_2, start=False, stop=True)
                s2col = rowp.tile([J, 1], BF, tag="s2col")
                nc.vector.tensor_copy(out=s2col, in_=ps2[0:J, 0:1])
                nc.tensor.transpose(tru[0:1, 2 * J:3 * J], s2col, ident16[0:J, 0:J])
                # rows 72/73 of uwq2 land via SBUF->SBUF DMA (engines cannot
                # write partition-offset APs; BIR verifier rejects them)
                s2q = rowp.tile([1, J], BF, tag="s2q")
                nc.vector.scalar_tensor_tensor(
                    out=s2q, in0=tru[0:1, 2 * J:3 * J], scalar=b_sb[0:1, 0:1],
                    in1=qm_b[0:1, :], op0=ADD, op1=MULT,
                )
                nc.gpsimd.dma_start(out=uwq2[K2:K2 + 1, :], in_=s2q)
                r100 = rowp.tile([1, J], BF, tag="r100")
                nc.vector.tensor_scalar_mul(
                    out=r100, in0=qm_b[0:1, :], scalar1=NEG_SOFT
                )
                nc.gpsimd.dma_start(out=uwq2[K2 + 1:K2 + 2, :], in_=r100)

                st["U16"], st["qm_b"], st["cm_t"] = U16, qm_b, cm_t
                st["uwq1"], st["uwq2"] = uwq1, uwq2
                st["maxes"] = rowp.tile([128, NCHUNK], FP, tag="maxes", name="maxes")
                st["rt16"] = rowp.tile([128, NCHUNK], BF, tag="rt16", name="rt16")
                st["rden"] = rowp.tile([128, NCHUNK], FP, tag="rden", name="rden")
                st["gq"] = [None] * NQUAD
                st["hbq"] = [None] * NQUAD
                st["ht"] = [None] * NPAIR
                st["ps_s"] = [None] * NPAIR
                st["e"] = [None] * NPAIR
                st["eT"] = [None] * NPAIR
                st["hbar"] = None
                return st

            def loadq(st, r, q):
                """One DMA issue: 4 chunks (512 t-rows) of H into a quad g tile."""
                t0 = q * 512
                gq = gp.tile([128, 4 * DG], FP, tag="g", name="gq")
                st["gq"][q] = gq
                gv = gq.rearrange("p (c x) -> p c x", c=4)
                nc.sync.dma_start(
                    out=gv[:, :, 0:D2],
                    in_=H_d[r, t0:t0 + 512, :].rearrange("(c p) d -> p c d", p=128),
                )

            def prepq(st, r, q):
                """bf16 copy of the quad's H + the two ones columns per chunk."""
                gq = st["gq"][q]
                hbq = chp.tile([128, 4 * WB], BF, tag="hb", bufs=3, name="hbq")
                st["hbq"][q] = hbq
                hv = hbq.rearrange("p (c x) -> p c x", c=4)
                gv = gq.rearrange("p (c x) -> p c x", c=4)
                nc.vector.tensor_copy(out=hv[:, :, 0:D2], in_=gv[:, :, 0:D2])
                nc.gpsimd.memset(hv[:, :, D2:WB], 1.0)

            def head(st, r, c):
                q, c4, h = c // 4, c % 4, c % 2
                hbq = st["hbq"][q]
                if h == 0:
                    st["trc"] = ps_trp.tile([128, 512], BF, tag="tr", name="trc")
                trc = st["trc"]
                o = c4 * WB
                nc.tensor.transpose(
                    trc[:, h * 256:h * 256 + 128], hbq[:, o:o + K1], ident16
                )
                nc.tensor.transpose(
                    trc[0:K2 + 2, h * 256 + 128:h * 256 + 256],
                    hbq[:, o + K1:o + WB], ident16,
                )
                if h == 1:
                    ht = chp.tile([128, 512], BF, tag="ht", bufs=2, name="ht")
                    st["ht"][c // 2] = ht
                    nc.scalar.copy(out=ht, in_=trc)

            def smm(st, r, c):
                p, h = c // 2, c % 2
                ht = st["ht"][p]
                if h == 0:
                    st["ps_s"][p] = ps_sp.tile([128, 128], FP, tag="s", name="ps_s")
                ps_s = st["ps_s"][p]
                nc.tensor.matmul(
                    ps_s[:, h * J:h * J + J], ht[:, h * 256:h * 256 + 128],
                    st["uwq1"], start=True, stop=False,
                )
                nc.tensor.matmul(
                    ps_s[:, h * J:h * J + J],
                    ht[0:K2 + 2, h * 256 + 128:h * 256 + 256],
                    st["uwq2"], start=False, stop=True,
                )

            def pair(st, r, p):
                c0, c1 = 2 * p, 2 * p + 1
                e_pair = chp.tile([128, 2 * J], BF, tag="e", bufs=2, name="e_pair")
                st["e"][p] = e_pair
                nc.scalar.activation(
                    out=e_pair, in_=st["ps_s"][p], func=EXP,
                    bias=negc[:, 0:1], scale=1.0,
                )
                ps_eT = ps_trp.tile([128, 128], BF, tag="tr", name="ps_eT")
                nc.tensor.transpose(ps_eT, e_pair, ident16)
                eT0 = chp.tile([J, 128], BF, tag="eT0", bufs=2, name="eT0")
                eT1 = chp.tile([J, 128], BF, tag="eT1", bufs=2, name="eT1")
                st["eT"][p] = (eT0, eT1)
                nc.vector.tensor_copy(out=eT0, in_=ps_eT[0:J, :])
                nc.scalar.copy(out=eT1, in_=ps_eT[J:2 * J, :])
                ep3 = e_pair.rearrange("p (k j) -> p k j", j=J)
                nc.vector.reduce_max(st["maxes"][:, c0:c1 + 1], ep3, axis=AXX)
                nc.vector.tensor_tensor(
                    out=st["rt16"][:, c0:c1 + 1], in0=st["maxes"][:, c0:c1 + 1],
                    in1=st["cm_t"][:, c0:c1 + 1], op=MULT,
                )
                if p == 0:
                    st["hbar"] = ps_hp.tile([1, D2 + 1], FP, tag="h", name="hbar")
                for c in (c0, c1):
                    q, c4 = c // 4, c % 4
                    nc.tensor.matmul(
                        st["hbar"], st["rt16"][:, c:c + 1],
                        st["hbq"][q][:, c4 * WB:c4 * WB + D2 + 1],
                        start=(c == 0), stop=(c == NCHUNK - 1),
                    )

            def tailp(st, r, p):
                c0 = 2 * p
                q, b0 = c0 // 4, (c0 % 4) * DG
                gq = st["gq"][q]
                ps_b = ps_up.tile([128, 2 * (D2 + 1)], FP, tag="u", name="ps_b")
                for h in (0, 1):
                    nc.tensor.matmul(
                        ps_b[:, h * (D2 + 1):(h + 1) * (D2 + 1)],
                        st["eT"][p][h], st["U16"], start=True, stop=True,
                    )
                # cols 200/401 = sum_j e = softmax denominators
                bv = ps_b.rearrange("p (c x) -> p c x", c=2)
                nc.vector.reciprocal(
                    out=st["rden"][:, c0:c0 + 2], in_=bv[:, :, D2:D2 + 1]
                )
                for h in (0, 1):
                    nc.scalar.activation(
                        out=gq[:, b0 + h * DG + D2:b0 + h * DG + 2 * D2],
                        in_=ps_b[:, h * (D2 + 1):h * (D2 + 1) + D2], func=COPYF,
                        scale=st["rden"][:, c0 + h:c0 + h + 1],
                    )
                # H*U_ for both chunks in one strided op
                gv2 = gq[:, b0:b0 + 2 * DG].rearrange("p (c x) -> p c x", c=2)
                nc.vector.tensor_tensor(
                    out=gv2[:, :, 2 * D2:3 * D2], in0=gv2[:, :, 0:D2],
                    in1=gv2[:, :, D2:2 * D2], op=MULT,
                )

            def rowend(st, r):
                # hbar col 200 = sum_t rt; normalize and broadcast over rows
                rs = rowp.tile([1, 1], FP, tag="rs")
                nc.vector.tensor_scalar_add(
                    out=rs, in0=st["hbar"][0:1, D2:D2 + 1], scalar1=1e-13
                )
                rs2 = rowp.tile([1, 1], FP, tag="rs2")
                nc.vector.reciprocal(out=rs2, in_=rs)
                hbar16 = rowp.tile([1, 2 * D2], BF, tag="hbar16")
                for h in (0, 1):
                    nc.vector.tensor_scalar_mul(
                        out=hbar16[:, h * D2:(h + 1) * D2],
                        in0=st["hbar"][0:1, 0:D2], scalar1=rs2[:, 0:1],
                    )
                ps_hb = ps_up.tile([128, 2 * D2], FP, tag="u", name="ps_hb")
                nc.tensor.matmul(
                    ps_hb, ones_row16, hbar16, start=True, stop=True
                )
                hb_sb = rowp.tile([128, 2 * D2], FP, tag="hb_sb")
                nc.vector.tensor_copy(out=hb_sb, in_=ps_hb)
                st["hb_sb"] = hb_sb

            def rowfinp(st, r, p):
                """H*H_ for pair p of a finished row, strided over both chunks."""
                c0 = 2 * p
                q, b0 = c0 // 4, (c0 % 4) * DG
                gq = st["gq"][q]
                gv2 = gq[:, b0:b0 + 2 * DG].rearrange("p (c x) -> p c x", c=2)
                hs2 = st["hb_sb"].rearrange("p (c x) -> p c x", c=2)
                nc.gpsimd.tensor_mul(
                    gv2[:, :, 3 * D2:4 * D2], gv2[:, :, 0:D2], hs2
                )

            def rowfinv(st, r, p):
                """rowfinp on the vector engine (final-row epilogue only)."""
                c0 = 2 * p
                q, b0 = c0 // 4, (c0 % 4) * DG
                gq = st["gq"][q]
                gv2 = gq[:, b0:b0 + 2 * DG].rearrange("p (c x) -> p c x", c=2)
                hs2 = st["hb_sb"].rearrange("p (c x) -> p c x", c=2)
                nc.vector.tensor_tensor(
                    out=gv2[:, :, 3 * D2:4 * D2], in0=gv2[:, :, 0:D2],
                    in1=hs2, op=MULT,
                )

            def storeq_early(st, r, q):
                """Store G cols 0:600 (H, U_, H*U_) as soon as the quad's
                tails are done -- overlaps the current row's compute."""
                t0 = q * 512
                gq = st["gq"][q]
                nc.sync.dma_start(
                    out=G_d[r, t0:t0 + 512, 0:3 * D2].rearrange(
                        "(c p) d -> p c d", p=128
                    ),
                    in_=gq.rearrange("p (c x) -> p c x", c=4)[:, :, 0:3 * D2],
                )

            def storeq_late(st, r, q):
                """Store G cols 600:800 (H*H_) after rowfin."""
                t0 = q * 512
                gq = st["gq"][q]
                nc.sync.dma_start(
                    out=G_d[r, t0:t0 + 512, 3 * D2:DG].rearrange(
                        "(c p) d -> p c d", p=128
                    ),
                    in_=gq.rearrange("p (c x) -> p c x", c=4)[:, :, 3 * D2:DG],
                )

            def headpair(st, r, p):
                head(st, r, 2 * p)
                head(st, r, 2 * p + 1)
                smm(st, r, 2 * p)
                smm(st, r, 2 * p + 1)

            # ---- cross-row pipelined schedule ----
            # H loads are issued before anything that could block the
            # in-order sync DMA queue; DMA issues are spread so every
            # pair-phase keeps the HBM engines fed.
            states = [None] * B
            states[0] = {"gq": [None] * NQUAD}
            loadq(states[0], 0, 0)
            loadq(states[0], 0, 1)
            gq0 = states[0]["gq"]
            states[0] = row_setup(0)
            states[0]["gq"] = gq0
            prepq(states[0], 0, 0)
            headpair(states[0], 0, 0)
            prepq(states[0], 0, 1)
            headpair(states[0], 0, 1)
            for r in range(B):
                st = states[r]
                prev = states[r - 1] if r > 0 else None
                nxt = None
                for p in range(NPAIR):
                    pair(st, r, p)
                    tailp(st, r, p)
                    if p == 0:
                        if r + 1 < B:
                            states[r + 1] = {"gq": [None] * NQUAD}
                            loadq(states[r + 1], r + 1, 0)
                        headpair(st, r, 2)
                        if prev is not None:
                            rowfinp(prev, r - 1, 0)
                            rowfinp(prev, r - 1, 1)
                            storeq_late(prev, r - 1, 0)
                    elif p == 1:
                        storeq_early(st, r, 0)
                        headpair(st, r, 3)
                        if r + 1 < B:
                            gqn = states[r + 1]["gq"]
                            states[r + 1] = row_setup(r + 1)
                            states[r + 1]["gq"] = gqn
                            nxt = states[r + 1]
                        if prev is not None:
                            rowfinp(prev, r - 1, 2)
                            rowfinp(prev, r - 1, 3)
                    elif p == 2:
                        if nxt is not None:
                            loadq(nxt, r + 1, 1)
                            prepq(nxt, r + 1, 0)
                            headpair(nxt, r + 1, 0)
                        if prev is not None:
                            storeq_late(prev, r - 1, 1)
                    elif p == 3:
                        storeq_early(st, r, 1)
                        if nxt is not None:
                            prepq(nxt, r + 1, 1)
                            headpair(nxt, r + 1, 1)
                rowend(st, r)
            last = states[B - 1]
            rowfinp(last, B - 1, 0)
            rowfinv(last, B - 1, 1)
            storeq_late(last, B - 1, 0)
            rowfinp(last, B - 1, 2)
            rowfinv(last, B - 1, 3)
            storeq_late(last, B - 1, 1)

    if split_waits:
        _split_overwide_waits(nc)
    return nc


_NC_CACHE = None


def _get_nc():
    global _NC_CACHE
    if _NC_CACHE is None:
        _NC_CACHE = build_program()
    return _NC_CACHE


def run_sharded(inputs, trace=False):
    from concourse.bass_utils import run_bass_kernel_spmd

    H = np.ascontiguousarray(np.asarray(inputs["H"], dtype=np.float32))
    U = np.ascontiguousarray(np.asarray(inputs["U"], dtype=np.float32))
    cm = np.ascontiguousarray(np.asarray(inputs["c_mask"], dtype=np.float32))
    qm = np.ascontiguousarray(np.asarray(inputs["q_mask"], dtype=np.float32))
    w = np.ascontiguousarray(np.asarray(inputs["w"], dtype=np.float32))
    b = np.asarray(inputs["b"], dtype=np.float32).reshape(1, 1)

    nc = _get_nc()
    in_maps = []
    for c in range(N_CORES):
        s = slice(c * B, (c + 1) * B)
        in_maps.append(
            {"H": H[s], "U": U[s], "c_mask": cm[s], "q_mask": qm[s], "w": w, "b": b}
        )
    res = run_bass_kernel_spmd(
        nc, in_maps, core_ids=list(range(N_CORES)), trace=trace
    )
    G = np.concatenate([res.results[c]["G"] for c in range(N_CORES)], axis=0)
    return G, res


def kernel(H, U, c_mask, q_mask, w, b):
    G, _ = run_sharded(
        {"H": H, "U": U, "c_mask": c_mask, "q_mask": q_mask, "w": w, "b": b}
    )
    return G
